# revision 9
# baseline (speedup 1.0000x reference)
"""Trainium2 Bass kernel for nn_AttentionMechanisms_1365799600322.

Reference computation (B=4, S=4096, HID=2048, H=16, D=128):
    q = x@Wq+bq; k = x@Wk+bk; v = x@Wv+bv          (reshaped [B,S,H,D])
    scores[b,s,h,g] = q[b,s,h,:]@k[b,s,g,:] * c_scale/sqrt(D)   # per-token HxH
    w = softmax(scores, -1); attn = w@v; out = attn@Wo + bo

Every op mixes only within a token, so we shard the B*S=16384 tokens
across 8 NeuronCores (2048 tokens/core) with zero collectives.

Per-core pipeline (bf16 matmul inputs, f32 PSUM accumulation), v2:
  xT [din, tok] passed transposed from host; weights pre-tiled on host so
  every weight-strip DMA is 4KB-contiguous per partition.
  qT/kT/vT [d, h*512+t]  head-major transposed projections; PSUM drains are
                         CONTIGUOUS (the (h,t) interleave needed by the
                         per-token HxH attention lives in strided operand
                         APs of the small 128x128 attention matmuls).
  v_grp [(g,t), d]       via PE transposes of vT 8-token column groups
                         (no DRAM bounce).
  scores s_T [(g,t),(h,t')] one 128x128 MM per 8-token group, 4 groups/bank
  p = exp(scale*s) * mask01   (ScalarE exp + DVE bf16 multiply; the 0/1
                               mask kills cross-token terms)
  rowsum <- all-ones stationary matmul (broadcasts col-sums to all parts)
  rinv   <- DVE reciprocal_approx_fast (single custom op)
  attn_T [d,(h,t')] <- lhsT=v_grp group, rhs=p group; normalize fused into
                       the PSUM->SBUF merge (writes head-major, 8-elem runs)
  outT [dout, tok]  <- lhsT=Wo block, rhs=attn_sb head slice (contiguous)
  Software pipeline: the attention batches of chunk c-1 are interleaved
  into the projection matmul stream of chunk c (one item after each
  16-matmul strip), so TensorE never sees a >1.5us gap and the HAM clock
  stays at 8/8.
  All four biases (when nonzero) enter as K=1 rank-1 matmuls appended to
  the PSUM accumulations (ones-row x bias-row), so no per-partition bias
  ops.
"""

import numpy as np
import ml_dtypes

import concourse.bass as bass
import concourse.mybir as mybir
from concourse.tile import TileContext

BF16 = mybir.dt.bfloat16
F32 = mybir.dt.float32

B, S, HID, H = 4, 4096, 2048, 16
D = HID // H            # 128
T_TOT = B * S           # 16384
NCORES = 8
T = T_TOT // NCORES     # 2048 tokens per core
KT = HID // 128         # 16 contraction tiles
FT = HID // 128         # 16 feature tiles (== heads for q/k/v layouts)
CHUNK = 512
NCHUNK = T // CHUNK     # 4
GRP = 8                 # tokens per attention group
NGRP = CHUNK // GRP     # 64 groups per chunk
QUAD = 4                # groups per psum batch
NBATCH = NGRP // QUAD   # 16 batches per chunk (32 tokens each)
STAG = 2                # pipeline stagger (slots) between attn stages


def _scalar_recip(nc, out, in_):
    """ScalarE Reciprocal unused; kept DVE reciprocal_approx_fast instead."""
    raise NotImplementedError


def build_nc(with_bias: bool = False):
    nc = bass.Bass()

    xT = nc.declare_dram_parameter("xT", [HID, T], BF16, isOutput=False)
    # pre-tiled: w_h[f*128+p, kt*128+m] = W[kt*128+p, f*128+m]
    wq = nc.declare_dram_parameter("wq", [HID, HID], BF16, isOutput=False)
    wk = nc.declare_dram_parameter("wk", [HID, HID], BF16, isOutput=False)
    wv = nc.declare_dram_parameter("wv", [HID, HID], BF16, isOutput=False)
    wo = nc.declare_dram_parameter("wo", [HID, HID], BF16, isOutput=False)
    brows = nc.declare_dram_parameter("brows", [1, 4 * HID], BF16, isOutput=False)
    scv = nc.declare_dram_parameter("scv", [128, 1], F32, isOutput=False)
    mask01 = nc.declare_dram_parameter("mask01", [128, 512], BF16, isOutput=False)
    ones = nc.declare_dram_parameter("ones", [128, 512], BF16, isOutput=False)
    ident = nc.declare_dram_parameter("ident", [128, 128], BF16, isOutput=False)
    outT = nc.declare_dram_parameter("outT", [HID, T], BF16, isOutput=True)

    xT_r = xT[:, :].rearrange("(kt p) t -> p kt t", p=128)       # [128,16,T]
    outT_r = outT[:, :].rearrange("(f p) t -> p f t", p=128)     # [128,16,T]

    with TileContext(nc) as tc:
        from contextlib import ExitStack

        with ExitStack() as _es:
            cpool = _es.enter_context(tc.tile_pool(name="const", bufs=1))
            xpool = _es.enter_context(tc.tile_pool(name="xin", bufs=2))
            qpool = _es.enter_context(tc.tile_pool(name="qq", bufs=2))
            kpool = _es.enter_context(tc.tile_pool(name="kk", bufs=2))
            vtpool = _es.enter_context(tc.tile_pool(name="vt", bufs=1))
            vgpool = _es.enter_context(tc.tile_pool(name="vgrp", bufs=2))
            atpool = _es.enter_context(tc.tile_pool(name="attnT", bufs=1))
            wpool = _es.enter_context(tc.tile_pool(name="wstrip", bufs=4))
            prpool = _es.enter_context(tc.tile_pool(name="praw", bufs=2))
            ppool = _es.enter_context(tc.tile_pool(name="pexp", bufs=5))
            rpool = _es.enter_context(tc.tile_pool(name="rinv", bufs=2))
            opool = _es.enter_context(tc.tile_pool(name="ostage", bufs=2))
            # PSUM: 4 pools x 2 banks = 8 banks
            projps = _es.enter_context(tc.tile_pool(name="pproj", bufs=2, space="PSUM"))
            scps = _es.enter_context(tc.tile_pool(name="psc", bufs=2, space="PSUM"))
            rsps = _es.enter_context(tc.tile_pool(name="prs", bufs=2, space="PSUM"))
            atps = _es.enter_context(tc.tile_pool(name="pat", bufs=2, space="PSUM"))

            # ---------------- constants ----------------
            br_sb = cpool.tile([1, 4 * HID], BF16, tag="br")   # bq,bk,bv,bo rows
            sc_sb = cpool.tile([128, 1], F32, tag="sc")
            mk_sb = cpool.tile([128, 512], BF16, tag="mask")
            ones_sb = cpool.tile([128, 512], BF16, tag="ones")
            id_sb = cpool.tile([128, 128], BF16, tag="ident")
            nc.sync.dma_start(out=br_sb[:], in_=brows[:, :])
            nc.sync.dma_start(out=sc_sb[:], in_=scv[:, :])
            nc.sync.dma_start(out=mk_sb[:], in_=mask01[:, :])
            nc.sync.dma_start(out=ones_sb[:], in_=ones[:, :])
            nc.sync.dma_start(out=id_sb[:], in_=ident[:, :])
            one_row = ones_sb[0:1, :]            # [1, 512] of ones
            ones_sq = ones_sb[:, 0:128]          # [128, 128] of ones

            # per-chunk live tiles
            x_sb = [None] * NCHUNK
            qT3 = [None] * NCHUNK
            kT3 = [None] * NCHUNK
            vT3 = [None] * NCHUNK
            vgr = [None] * NCHUNK
            at_sb = [None] * NCHUNK
            p_tiles = {}
            r_tiles = {}

            def load_x(c):
                t0 = c * CHUNK
                x_sb[c] = xpool.tile([128, KT * CHUNK], BF16, tag="x", name=f"x{c}")
                nc.sync.dma_start(
                    out=x_sb[c][:].rearrange("p (kt t) -> p kt t", t=CHUNK),
                    in_=xT_r[:, :, t0 : t0 + CHUNK],
                )

            def proj_strip(c, w_h, bidx, dst4, f, eng):
                """Transposed projection strip f of a group-major [d,(g,h,t)] dst."""
                w_sb = wpool.tile([128, KT * 128], BF16, tag="w", name=f"w{c}_{f}")
                nc.sync.dma_start(out=w_sb[:], in_=w_h[f * 128 : (f + 1) * 128, :])
                ps = projps.tile([128, CHUNK], F32, tag="pp", name=f"pp{c}_{f}")
                for kt in range(KT):
                    nc.tensor.matmul(
                        ps[:],
                        lhsT=w_sb[:, kt * 128 : (kt + 1) * 128],
                        rhs=x_sb[c][:, kt * CHUNK : (kt + 1) * CHUNK],
                        start=(kt == 0),
                        stop=(not with_bias and kt == KT - 1),
                    )
                if with_bias:
                    nc.tensor.matmul(
                        ps[:],
                        lhsT=br_sb[0:1, bidx * HID + f * 128 : bidx * HID + (f + 1) * 128],
                        rhs=one_row,
                        start=False,
                        stop=True,
                    )
                ps3 = ps[:].rearrange("p (g t) -> p g t", t=GRP)
                # group-major drain: dst col = g2*128 + f*8 + t (8-elem runs)
                if eng == "act":
                    nc.scalar.copy(out=dst4[:, :, f, :], in_=ps3)
                else:
                    nc.vector.tensor_copy(out=dst4[:, :, f, :], in_=ps3)

            def o_strip(c, f):
                """Output projection strip f for chunk c (reads at_sb[c])."""
                t0 = c * CHUNK
                w_sb = wpool.tile([128, KT * 128], BF16, tag="w", name=f"wo{c}_{f}")
                nc.sync.dma_start(out=w_sb[:], in_=wo[f * 128 : (f + 1) * 128, :])
                ps = projps.tile([128, CHUNK], F32, tag="pp", name=f"ppo{c}_{f}")
                for kt in range(KT):
                    nc.tensor.matmul(
                        ps[:],
                        lhsT=w_sb[:, kt * 128 : (kt + 1) * 128],
                        rhs=at_sb[c][:, kt * CHUNK : (kt + 1) * CHUNK],
                        start=(kt == 0),
                        stop=(not with_bias and kt == KT - 1),
                    )
                if with_bias:
                    nc.tensor.matmul(
                        ps[:],
                        lhsT=br_sb[0:1, 3 * HID + f * 128 : 3 * HID + (f + 1) * 128],
                        rhs=one_row,
                        start=False,
                        stop=True,
                    )
                o_sb = opool.tile([128, CHUNK], BF16, tag="o", name=f"o{c}_{f}")
                nc.scalar.copy(out=o_sb[:], in_=ps[:])
                nc.sync.dma_start(out=outT_r[:, f, t0 : t0 + CHUNK], in_=o_sb[:])

            def transp_quad(c, j):
                """PE-transpose groups 4j..4j+3 of vT into v_grp [(g,t), d]."""
                ps = projps.tile([128, 512], BF16, tag="pp", name=f"ptr{c}_{j}")
                for i in range(4):
                    g2 = 4 * j + i
                    nc.tensor.transpose(
                        ps[:, i * 128 : (i + 1) * 128],
                        in_=vT3[c][0][:, g2 * 128 : (g2 + 1) * 128],
                        identity=id_sb[:],
                    )
                nc.vector.tensor_copy(
                    out=vgr[c][:, j * 512 : (j + 1) * 512], in_=ps[:]
                )

            def scores_item(c, b):
                ps = scps.tile([128, 512], F32, tag="sc", name=f"sc{c}_{b}")
                for q in range(QUAD):
                    g2 = b * 4 + q
                    nc.tensor.matmul(
                        ps[:, q * 128 : (q + 1) * 128],
                        lhsT=kT3[c][0][:, g2 * 128 : (g2 + 1) * 128],
                        rhs=qT3[c][0][:, g2 * 128 : (g2 + 1) * 128],
                        start=True,
                        stop=True,
                    )
                praw = prpool.tile([128, 512], BF16, tag="praw", name=f"pr{c}_{b}")
                nc.scalar.activation(
                    out=praw[:], in_=ps[:],
                    func=mybir.ActivationFunctionType.Exp,
                    scale=sc_sb[:, 0:1],
                )
                p_sb = ppool.tile([128, 512], BF16, tag="p", name=f"p{c}_{b}")
                nc.vector.tensor_tensor(
                    out=p_sb[:], in0=praw[:], in1=mk_sb[:],
                    op=mybir.AluOpType.mult,
                )
                p_tiles[(c, b)] = p_sb

            def rowsum_item(c, b):
                p_sb = p_tiles[(c, b)]
                prs = rsps.tile([128, 512], F32, tag="rs", name=f"rs{c}_{b}")
                nc.tensor.matmul(
                    prs[:], lhsT=ones_sq, rhs=p_sb[:], start=True, stop=True
                )
                rinv = rpool.tile([128, 512], F32, tag="ri", name=f"ri{c}_{b}")
                # DVE reciprocal: 3.3us but keeps ScalarE mono-function (Exp);
                # alternating Exp/Reciprocal on ACT costs a 1.3us table load
                # per switch.
                nc.vector.reciprocal(out=rinv[:], in_=prs[:])
                r_tiles[(c, b)] = rinv

            def attn_item(c, b):
                p_sb = p_tiles.pop((c, b))
                rinv = r_tiles.pop((c, b))
                pat = atps.tile([128, 512], F32, tag="at", name=f"pat{c}_{b}")
                for q in range(QUAD):
                    g2 = b * 4 + q
                    nc.tensor.matmul(
                        pat[:, q * 128 : (q + 1) * 128],
                        lhsT=vgr[c][:, g2 * 128 : (g2 + 1) * 128],
                        rhs=p_sb[:, q * 128 : (q + 1) * 128],
                        start=True,
                        stop=True,
                    )
                # normalized merge into head-major attn_sb: psum cols are
                # (q, h, t); dst col = h*512 + b*32 + q*8 + t  (8-elem runs)
                at_hm = at_sb[c][:].rearrange("p (h t) -> p h t", t=CHUNK)
                dst = at_hm[:, :, b * 32 : (b + 1) * 32].rearrange(
                    "p h (q t) -> p q h t", t=GRP
                )
                nc.vector.tensor_tensor(
                    out=dst,
                    in0=pat[:].rearrange("p (q h t) -> p q h t", h=H, t=GRP),
                    in1=rinv[:].rearrange("p (q h t) -> p q h t", h=H, t=GRP),
                    op=mybir.AluOpType.mult,
                )

            def attn_items_staggered(c):
                items = []
                for i in range(NBATCH + 2 * STAG):
                    if i < NBATCH:
                        items.append(lambda b=i: scores_item(c, b))
                    if STAG <= i < NBATCH + STAG:
                        items.append(lambda b=i - STAG: rowsum_item(c, b))
                    if 2 * STAG <= i:
                        items.append(lambda b=i - 2 * STAG: attn_item(c, b))
                return items

            # ================= main pipeline =================
            load_x(0)
            for c in range(NCHUNK):
                if c + 1 < NCHUNK:
                    load_x(c + 1)
                qT_sb = qpool.tile([128, H * CHUNK], BF16, tag="qT", name=f"qT{c}")
                kT_sb = kpool.tile([128, H * CHUNK], BF16, tag="kT", name=f"kT{c}")
                vT_sb = vtpool.tile([128, H * CHUNK], BF16, tag="vT", name=f"vT{c}")
                # group-major: col = g2*128 + h*8 + t
                qT3[c] = (qT_sb[:],
                          qT_sb[:].rearrange("p (g h t) -> p g h t", h=H, t=GRP))
                kT3[c] = (kT_sb[:],
                          kT_sb[:].rearrange("p (g h t) -> p g h t", h=H, t=GRP))
                vT3[c] = (vT_sb[:],
                          vT_sb[:].rearrange("p (g h t) -> p g h t", h=H, t=GRP))
                vgr[c] = vgpool.tile([128, NGRP * 128], BF16, tag="vg", name=f"vg{c}")
                at_sb[c] = atpool.tile([128, H * CHUNK], BF16, tag="at", name=f"at{c}")

                # 48 projection strips with attn(c-1) interleaved
                pend = attn_items_staggered(c - 1) if c > 0 else []
                slots = []
                for f in range(FT):
                    slots.append(lambda f=f: proj_strip(c, wv, 2, vT3[c][1], f, "act"))
                for f in range(FT):
                    slots.append(lambda f=f: proj_strip(c, wq, 0, qT3[c][1], f, "act"))
                for f in range(FT):
                    slots.append(lambda f=f: proj_strip(c, wk, 1, kT3[c][1], f, "act"))
                acc = 0.0
                rate = len(pend) / len(slots) if slots else 0.0
                for s in slots:
                    s()
                    acc += rate
                    while acc >= 1.0 and pend:
                        pend.pop(0)()
                        acc -= 1.0
                while pend:
                    pend.pop(0)()

                # O-proj of chunk c-1 with transposes of chunk c interleaved;
                # on the last chunk also drain its own attention here so the
                # epilogue is a dense O-proj instead of a latency-bound chain.
                pend2 = [lambda j=j: transp_quad(c, j) for j in range(NBATCH)]
                if c == NCHUNK - 1:
                    pend2 += attn_items_staggered(c)
                if c > 0:
                    acc2 = 0.0
                    rate2 = len(pend2) / FT
                    for f in range(FT):
                        o_strip(c - 1, f)
                        acc2 += rate2
                        while acc2 >= 1.0 and pend2:
                            pend2.pop(0)()
                            acc2 -= 1.0
                while pend2:
                    pend2.pop(0)()

            # epilogue: O-proj of the last chunk (attention already drained)
            for f in range(FT):
                o_strip(NCHUNK - 1, f)

    return nc


# Opcodes whose encodings accept multiple sync waits. On TRN2 every TPB
# engine instruction (and the DMA pseudo-instruction) takes at most ONE
# wait, so surplus waits are split into standalone EventSemaphore
# instructions spliced just before the offender (same engine stream =>
# identical semantics).
_WAIT_BUDGET = {}


def _split_waits_json(bir: bytes) -> bytes:
    import orjson

    j = orjson.loads(bir)
    ctr = 0
    for fn in j["functions"]:
        for blk in fn["blocks"]:
            out = []
            for ins in blk["instructions"]:
                si = ins.get("sync_info")
                waits = (si or {}).get("on_wait") or []
                budget = _WAIT_BUDGET.get(ins.get("opcode"), 1)
                if len(waits) > budget:
                    for w in waits[:-budget]:
                        ctr += 1
                        out.append(
                            {
                                "debug": ins.get("debug", 0),
                                "engine": ins["engine"],
                                "ins": [],
                                "name": f"Wsplit-{ctr}",
                                "opcode": "EventSemaphore",
                                "outs": [],
                                "sync_info": {"on_update": [], "on_wait": [w]},
                            }
                        )
                    si["on_wait"] = waits[-budget:]
                out.append(ins)
            blk["instructions"] = out
    return orjson.dumps(j)


def _install_ntff_shim():
    """This image's antenv lacks axon_hooks; provide it so trace=True works."""
    import sys, types

    if "antenv.axon_hooks" in sys.modules:
        return
    mod = types.ModuleType("antenv.axon_hooks")
    mod._hook = None

    def set_axon_ntff_profile_hook(h):
        mod._hook = h

    def get_axon_ntff_profile_hook():
        return mod._hook

    mod.set_axon_ntff_profile_hook = set_axon_ntff_profile_hook
    mod.get_axon_ntff_profile_hook = get_axon_ntff_profile_hook
    sys.modules["antenv.axon_hooks"] = mod
    try:
        import antenv

        antenv.axon_hooks = mod
    except ImportError:
        pass
    try:
        from trn_agent_boot.trn_boot import _ntff_profile_via_ctypes

        mod.set_axon_ntff_profile_hook(
            _ntff_profile_via_ctypes("/opt/axon/libaxon_pjrt.so")
        )
    except Exception as e:  # degrade: tracing skipped, run still works
        print(f"ntff shim: hook registration failed: {e}")


def _host_inputs(x, Wq, bq, Wk, bk, Wv, bv, Wo, bo, c_scale):
    """Build per-core in_maps (host-side shard + transpose + bf16 cast)."""
    bf = ml_dtypes.bfloat16
    xf = np.ascontiguousarray(np.asarray(x, np.float32).reshape(T_TOT, HID))

    def tile_w(W):  # w_h[f*128+p, kt*128+m] = W[kt*128+p, f*128+m]
        Wb = np.asarray(W, np.float32).astype(bf)
        return np.ascontiguousarray(
            Wb.reshape(KT, 128, FT, 128).transpose(2, 1, 0, 3).reshape(HID, HID)
        )

    brows = np.concatenate(
        [np.asarray(v, np.float32) for v in (bq, bk, bv, bo)]
    ).astype(bf).reshape(1, 4 * HID)

    scale = float(np.asarray(c_scale, np.float32).reshape(-1)[0]) / np.sqrt(D)
    scv = np.full((128, 1), scale, np.float32)

    # mask01[g*8+t, q*128 + h*8 + t'] = 1 if t==t' else 0
    m = np.zeros((128, 512), bf)
    for t in range(GRP):
        for q in range(QUAD):
            for h in range(H):
                m[np.arange(H) * GRP + t, q * 128 + h * GRP + t] = 1
    ones_b = np.ones((128, 512), bf)
    ident = np.eye(128, dtype=bf)

    shared = dict(
        wq=tile_w(Wq), wk=tile_w(Wk), wo=tile_w(Wo), wv=tile_w(Wv),
        brows=brows, scv=scv, mask01=m, ones=ones_b, ident=ident,
    )
    in_maps = []
    for i in range(NCORES):
        xT_i = np.ascontiguousarray(xf[i * T : (i + 1) * T].T.astype(bf))
        in_maps.append(dict(xT=xT_i, **shared))
    return in_maps


def _assemble(results):
    outs = []
    for i in range(NCORES):
        outs.append(np.asarray(results[i]["outT"], np.float32).T)  # [T, HID]
    return np.concatenate(outs, axis=0).reshape(B, S, HID)


def run(inputs: dict, trace: bool = False):
    """Compile + execute on 8 cores; returns (output, BassKernelResults)."""
    from concourse.bass_utils import run_bass_kernel_spmd

    if trace:
        _install_ntff_shim()
    wb = any(
        np.any(np.asarray(inputs[k], np.float32) != 0.0)
        for k in ("bq", "bk", "bv", "bo")
    )
    nc = build_nc(with_bias=wb)
    _orig_tjb = nc.to_json_bytes
    nc.to_json_bytes = lambda: _split_waits_json(_orig_tjb())
    in_maps = _host_inputs(**inputs)
    res = run_bass_kernel_spmd(
        nc, in_maps, core_ids=list(range(NCORES)), trace=trace
    )
    return _assemble(res.results), res


def kernel(**inputs) -> np.ndarray:
    out, _ = run(inputs, trace=False)
    return out


# revision 10
# speedup vs baseline: 1.0329x; 1.0329x over previous
"""Trainium2 Bass kernel for nn_AttentionMechanisms_1365799600322.

Reference computation (B=4, S=4096, HID=2048, H=16, D=128):
    q = x@Wq+bq; k = x@Wk+bk; v = x@Wv+bv          (reshaped [B,S,H,D])
    scores[b,s,h,g] = q[b,s,h,:]@k[b,s,g,:] * c_scale/sqrt(D)   # per-token HxH
    w = softmax(scores, -1); attn = w@v; out = attn@Wo + bo

Every op mixes only within a token, so we shard the B*S=16384 tokens
across 8 NeuronCores (2048 tokens/core) with zero collectives.

Per-core pipeline (bf16 matmul inputs, f32 PSUM accumulation), v2:
  xT [din, tok] passed transposed from host; weights pre-tiled on host so
  every weight-strip DMA is 4KB-contiguous per partition.
  qT/kT/vT [d, h*512+t]  head-major transposed projections; PSUM drains are
                         CONTIGUOUS (the (h,t) interleave needed by the
                         per-token HxH attention lives in strided operand
                         APs of the small 128x128 attention matmuls).
  v_grp [(g,t), d]       via PE transposes of vT 8-token column groups
                         (no DRAM bounce).
  scores s_T [(g,t),(h,t')] one 128x128 MM per 8-token group, 4 groups/bank
  p = exp(scale*s) * mask01   (ScalarE exp + DVE bf16 multiply; the 0/1
                               mask kills cross-token terms)
  rowsum <- all-ones stationary matmul (broadcasts col-sums to all parts)
  rinv   <- DVE reciprocal_approx_fast (single custom op)
  attn_T [d,(h,t')] <- lhsT=v_grp group, rhs=p group; normalize fused into
                       the PSUM->SBUF merge (writes head-major, 8-elem runs)
  outT [dout, tok]  <- lhsT=Wo block, rhs=attn_sb head slice (contiguous)
  Software pipeline: the attention batches of chunk c-1 are interleaved
  into the projection matmul stream of chunk c (one item after each
  16-matmul strip), so TensorE never sees a >1.5us gap and the HAM clock
  stays at 8/8.
  All four biases (when nonzero) enter as K=1 rank-1 matmuls appended to
  the PSUM accumulations (ones-row x bias-row), so no per-partition bias
  ops.
"""

import numpy as np
import ml_dtypes

import concourse.bass as bass
import concourse.mybir as mybir
from concourse.tile import TileContext

BF16 = mybir.dt.bfloat16
F32 = mybir.dt.float32

B, S, HID, H = 4, 4096, 2048, 16
D = HID // H            # 128
T_TOT = B * S           # 16384
NCORES = 8
T = T_TOT // NCORES     # 2048 tokens per core
KT = HID // 128         # 16 contraction tiles
FT = HID // 128         # 16 feature tiles (== heads for q/k/v layouts)
CHUNK = 512
NCHUNK = T // CHUNK     # 4
GRP = 8                 # tokens per attention group
NGRP = CHUNK // GRP     # 64 groups per chunk
QUAD = 4                # groups per psum batch
NBATCH = NGRP // QUAD   # 16 batches per chunk (32 tokens each)
STAG = 2                # pipeline stagger (slots) between attn stages


def _scalar_recip(nc, out, in_):
    """ScalarE Reciprocal unused; kept DVE reciprocal_approx_fast instead."""
    raise NotImplementedError


def build_nc(with_bias: bool = False):
    nc = bass.Bass()

    xT = nc.declare_dram_parameter("xT", [HID, T], BF16, isOutput=False)
    # pre-tiled: w_h[f*128+p, kt*128+m] = W[kt*128+p, f*128+m]
    wq = nc.declare_dram_parameter("wq", [HID, HID], BF16, isOutput=False)
    wk = nc.declare_dram_parameter("wk", [HID, HID], BF16, isOutput=False)
    wv = nc.declare_dram_parameter("wv", [HID, HID], BF16, isOutput=False)
    wo = nc.declare_dram_parameter("wo", [HID, HID], BF16, isOutput=False)
    brows = nc.declare_dram_parameter("brows", [1, 4 * HID], BF16, isOutput=False)
    scv = nc.declare_dram_parameter("scv", [128, 1], F32, isOutput=False)
    mrows = nc.declare_dram_parameter("mrows", [9, 128], BF16, isOutput=False)
    mcols = nc.declare_dram_parameter("mcols", [9, 512], BF16, isOutput=False)
    ones = nc.declare_dram_parameter("ones", [128, 512], BF16, isOutput=False)
    ident = nc.declare_dram_parameter("ident", [128, 128], BF16, isOutput=False)
    outT = nc.declare_dram_parameter("outT", [HID, T], BF16, isOutput=True)

    xT_r = xT[:, :].rearrange("(kt p) t -> p kt t", p=128)       # [128,16,T]
    outT_r = outT[:, :].rearrange("(f p) t -> p f t", p=128)     # [128,16,T]

    with TileContext(nc) as tc:
        from contextlib import ExitStack

        with ExitStack() as _es:
            cpool = _es.enter_context(tc.tile_pool(name="const", bufs=1))
            xpool = _es.enter_context(tc.tile_pool(name="xin", bufs=2))
            qpool = _es.enter_context(tc.tile_pool(name="qq", bufs=2))
            kpool = _es.enter_context(tc.tile_pool(name="kk", bufs=2))
            vtpool = _es.enter_context(tc.tile_pool(name="vt", bufs=1))
            vgpool = _es.enter_context(tc.tile_pool(name="vgrp", bufs=2))
            atpool = _es.enter_context(tc.tile_pool(name="attnT", bufs=1))
            wpool = _es.enter_context(tc.tile_pool(name="wstrip", bufs=4))
            prpool = _es.enter_context(tc.tile_pool(name="praw", bufs=2))
            pnpool = _es.enter_context(tc.tile_pool(name="pnorm", bufs=3))
            ppool = _es.enter_context(tc.tile_pool(name="pgrp", bufs=3))
            rpool = _es.enter_context(tc.tile_pool(name="rsum", bufs=3))
            opool = _es.enter_context(tc.tile_pool(name="ostage", bufs=2))
            # PSUM: 4 pools x 2 banks = 8 banks
            projps = _es.enter_context(tc.tile_pool(name="pproj", bufs=2, space="PSUM"))
            scps = _es.enter_context(tc.tile_pool(name="psc", bufs=2, space="PSUM"))
            ptps = _es.enter_context(tc.tile_pool(name="ppt", bufs=2, space="PSUM"))
            atps = _es.enter_context(tc.tile_pool(name="pat", bufs=2, space="PSUM"))

            # ---------------- constants ----------------
            br_sb = cpool.tile([1, 4 * HID], BF16, tag="br")   # bq,bk,bv,bo rows
            sc_sb = cpool.tile([128, 1], F32, tag="sc")
            mr_sb = cpool.tile([9, 128], BF16, tag="mrow")
            mc_sb = cpool.tile([9, 512], BF16, tag="mcol")
            ones_sb = cpool.tile([128, 512], BF16, tag="ones")
            id_sb = cpool.tile([128, 128], BF16, tag="ident")
            nc.sync.dma_start(out=br_sb[:], in_=brows[:, :])
            nc.sync.dma_start(out=sc_sb[:], in_=scv[:, :])
            nc.sync.dma_start(out=mr_sb[:], in_=mrows[:, :])
            nc.sync.dma_start(out=mc_sb[:], in_=mcols[:, :])
            nc.sync.dma_start(out=ones_sb[:], in_=ones[:, :])
            nc.sync.dma_start(out=id_sb[:], in_=ident[:, :])
            one_row = ones_sb[0:1, :]            # [1, 512] of ones
            ones_sq = ones_sb[:, 0:128]          # [128, 128] of ones

            # per-chunk live tiles
            x_sb = [None] * NCHUNK
            qT3 = [None] * NCHUNK
            kT3 = [None] * NCHUNK
            vT3 = [None] * NCHUNK
            vgr = [None] * NCHUNK
            at_sb = [None] * NCHUNK
            p_tiles = {}

            def load_x(c):
                t0 = c * CHUNK
                x_sb[c] = xpool.tile([128, KT * CHUNK], BF16, tag="x", name=f"x{c}")
                nc.sync.dma_start(
                    out=x_sb[c][:].rearrange("p (kt t) -> p kt t", t=CHUNK),
                    in_=xT_r[:, :, t0 : t0 + CHUNK],
                )

            def proj_strip(c, w_h, bidx, dst4, f, eng):
                """Transposed projection strip f of a group-major [d,(g,h,t)] dst."""
                w_sb = wpool.tile([128, KT * 128], BF16, tag="w", name=f"w{c}_{f}")
                nc.sync.dma_start(out=w_sb[:], in_=w_h[f * 128 : (f + 1) * 128, :])
                ps = projps.tile([128, CHUNK], F32, tag="pp", name=f"pp{c}_{f}")
                for kt in range(KT):
                    nc.tensor.matmul(
                        ps[:],
                        lhsT=w_sb[:, kt * 128 : (kt + 1) * 128],
                        rhs=x_sb[c][:, kt * CHUNK : (kt + 1) * CHUNK],
                        start=(kt == 0),
                        stop=(not with_bias and kt == KT - 1),
                    )
                if with_bias:
                    nc.tensor.matmul(
                        ps[:],
                        lhsT=br_sb[0:1, bidx * HID + f * 128 : bidx * HID + (f + 1) * 128],
                        rhs=one_row,
                        start=False,
                        stop=True,
                    )
                ps3 = ps[:].rearrange("p (g t) -> p g t", t=GRP)
                # group-major drain: dst col = g2*128 + f*8 + t (8-elem runs)
                if eng == "act":
                    nc.scalar.copy(out=dst4[:, :, f, :], in_=ps3)
                else:
                    nc.vector.tensor_copy(out=dst4[:, :, f, :], in_=ps3)

            def o_strip(c, f):
                """Output projection strip f for chunk c (reads at_sb[c])."""
                t0 = c * CHUNK
                w_sb = wpool.tile([128, KT * 128], BF16, tag="w", name=f"wo{c}_{f}")
                nc.sync.dma_start(out=w_sb[:], in_=wo[f * 128 : (f + 1) * 128, :])
                ps = projps.tile([128, CHUNK], F32, tag="pp", name=f"ppo{c}_{f}")
                for kt in range(KT):
                    nc.tensor.matmul(
                        ps[:],
                        lhsT=w_sb[:, kt * 128 : (kt + 1) * 128],
                        rhs=at_sb[c][:, kt * CHUNK : (kt + 1) * CHUNK],
                        start=(kt == 0),
                        stop=(not with_bias and kt == KT - 1),
                    )
                if with_bias:
                    nc.tensor.matmul(
                        ps[:],
                        lhsT=br_sb[0:1, 3 * HID + f * 128 : 3 * HID + (f + 1) * 128],
                        rhs=one_row,
                        start=False,
                        stop=True,
                    )
                o_sb = opool.tile([128, CHUNK], BF16, tag="o", name=f"o{c}_{f}")
                nc.scalar.copy(out=o_sb[:], in_=ps[:])
                nc.sync.dma_start(out=outT_r[:, f, t0 : t0 + CHUNK], in_=o_sb[:])

            def transp_quad(c, j):
                """PE-transpose groups 4j..4j+3 of vT into v_grp [(g,t), d]."""
                ps = projps.tile([128, 512], BF16, tag="pp", name=f"ptr{c}_{j}")
                for i in range(4):
                    g2 = 4 * j + i
                    nc.tensor.transpose(
                        ps[:, i * 128 : (i + 1) * 128],
                        in_=vT3[c][0][:, g2 * 128 : (g2 + 1) * 128],
                        identity=id_sb[:],
                    )
                nc.vector.tensor_copy(
                    out=vgr[c][:, j * 512 : (j + 1) * 512], in_=ps[:]
                )

            def scores_item(c, b):
                # scores with softmax rows in PARTITIONS: out[(h,t'),(g,t)]
                # = q . k + additive rank-9 mask (-1e5 off the t==t' diagonal)
                ps = scps.tile([128, 512], F32, tag="sc", name=f"sc{c}_{b}")
                nc.tensor.matmul(ps[:], lhsT=mr_sb[:], rhs=mc_sb[:],
                                 start=True, stop=False)
                for q in range(QUAD):
                    g2 = b * 4 + q
                    nc.tensor.matmul(
                        ps[:, q * 128 : (q + 1) * 128],
                        lhsT=qT3[c][0][:, g2 * 128 : (g2 + 1) * 128],
                        rhs=kT3[c][0][:, g2 * 128 : (g2 + 1) * 128],
                        start=False,
                        stop=(q == QUAD - 1),
                        skip_group_check=True,
                    )
                # per-quad exp with free per-partition rowsum accumulation
                praw = prpool.tile([128, 512], F32, tag="praw", name=f"pr{c}_{b}")
                rsum = rpool.tile([128, 2 * QUAD], F32, tag="rs", name=f"rs{c}_{b}")
                for q in range(QUAD):
                    nc.scalar.activation(
                        out=praw[:, q * 128 : (q + 1) * 128],
                        in_=ps[:, q * 128 : (q + 1) * 128],
                        func=mybir.ActivationFunctionType.Exp,
                        scale=sc_sb[:, 0:1],
                        accum_out=rsum[:, q : q + 1],
                    )
                nc.vector.reciprocal(out=rsum[:, QUAD : 2 * QUAD],
                                     in_=rsum[:, 0:QUAD])
                pnorm = pnpool.tile([128, 512], BF16, tag="pn", name=f"pn{c}_{b}")
                for q in range(QUAD):
                    nc.vector.tensor_scalar(
                        out=pnorm[:, q * 128 : (q + 1) * 128],
                        in0=praw[:, q * 128 : (q + 1) * 128],
                        scalar1=rsum[:, QUAD + q : QUAD + q + 1],
                        scalar2=None,
                        op0=mybir.AluOpType.mult,
                    )
                p_tiles[(c, b)] = pnorm

            def ptransp_item(c, b):
                # PE-transpose normalized weights back to [(g,t), (h,t')]
                pnorm = p_tiles.pop((c, b))
                pt = ptps.tile([128, 512], BF16, tag="pt", name=f"pt{c}_{b}")
                for q in range(QUAD):
                    nc.tensor.transpose(
                        pt[:, q * 128 : (q + 1) * 128],
                        in_=pnorm[:, q * 128 : (q + 1) * 128],
                        identity=id_sb[:],
                    )
                p_grp = ppool.tile([128, 512], BF16, tag="pg", name=f"pg{c}_{b}")
                nc.vector.tensor_copy(out=p_grp[:], in_=pt[:])
                p_tiles[(c, b, "g")] = p_grp

            def attn_item(c, b):
                p_grp = p_tiles.pop((c, b, "g"))
                pat = atps.tile([128, 512], F32, tag="at", name=f"pat{c}_{b}")
                for q in range(QUAD):
                    g2 = b * 4 + q
                    nc.tensor.matmul(
                        pat[:, q * 128 : (q + 1) * 128],
                        lhsT=vgr[c][:, g2 * 128 : (g2 + 1) * 128],
                        rhs=p_grp[:, q * 128 : (q + 1) * 128],
                        start=True,
                        stop=True,
                    )
                # merge into head-major attn_sb: psum cols are (q, h, t);
                # dst col = h*512 + b*32 + q*8 + t  (8-elem runs)
                at_hm = at_sb[c][:].rearrange("p (h t) -> p h t", t=CHUNK)
                dst = at_hm[:, :, b * 32 : (b + 1) * 32].rearrange(
                    "p h (q t) -> p q h t", t=GRP
                )
                nc.vector.tensor_copy(
                    out=dst,
                    in_=pat[:].rearrange("p (q h t) -> p q h t", h=H, t=GRP),
                )

            def attn_items_staggered(c):
                items = []
                for i in range(NBATCH + 2 * STAG):
                    if i < NBATCH:
                        items.append(lambda b=i: scores_item(c, b))
                    if STAG <= i < NBATCH + STAG:
                        items.append(lambda b=i - STAG: ptransp_item(c, b))
                    if 2 * STAG <= i:
                        items.append(lambda b=i - 2 * STAG: attn_item(c, b))
                return items

            # ================= main pipeline =================
            load_x(0)
            for c in range(NCHUNK):
                if c + 1 < NCHUNK:
                    load_x(c + 1)
                qT_sb = qpool.tile([128, H * CHUNK], BF16, tag="qT", name=f"qT{c}")
                kT_sb = kpool.tile([128, H * CHUNK], BF16, tag="kT", name=f"kT{c}")
                vT_sb = vtpool.tile([128, H * CHUNK], BF16, tag="vT", name=f"vT{c}")
                # group-major: col = g2*128 + h*8 + t
                qT3[c] = (qT_sb[:],
                          qT_sb[:].rearrange("p (g h t) -> p g h t", h=H, t=GRP))
                kT3[c] = (kT_sb[:],
                          kT_sb[:].rearrange("p (g h t) -> p g h t", h=H, t=GRP))
                vT3[c] = (vT_sb[:],
                          vT_sb[:].rearrange("p (g h t) -> p g h t", h=H, t=GRP))
                vgr[c] = vgpool.tile([128, NGRP * 128], BF16, tag="vg", name=f"vg{c}")
                at_sb[c] = atpool.tile([128, H * CHUNK], BF16, tag="at", name=f"at{c}")

                # 48 projection strips with attn(c-1) interleaved
                pend = attn_items_staggered(c - 1) if c > 0 else []
                slots = []
                for f in range(FT):
                    slots.append(lambda f=f: proj_strip(c, wv, 2, vT3[c][1], f, "act"))
                for f in range(FT):
                    slots.append(lambda f=f: proj_strip(c, wq, 0, qT3[c][1], f, "act"))
                for f in range(FT):
                    slots.append(lambda f=f: proj_strip(c, wk, 1, kT3[c][1], f, "act"))
                acc = 0.0
                rate = len(pend) / len(slots) if slots else 0.0
                for s in slots:
                    s()
                    acc += rate
                    while acc >= 1.0 and pend:
                        pend.pop(0)()
                        acc -= 1.0
                while pend:
                    pend.pop(0)()

                # O-proj of chunk c-1 with transposes of chunk c interleaved;
                # on the last chunk also drain its own attention here so the
                # epilogue is a dense O-proj instead of a latency-bound chain.
                pend2 = [lambda j=j: transp_quad(c, j) for j in range(NBATCH)]
                if c == NCHUNK - 1:
                    pend2 += attn_items_staggered(c)
                if c > 0:
                    acc2 = 0.0
                    rate2 = len(pend2) / FT
                    for f in range(FT):
                        o_strip(c - 1, f)
                        acc2 += rate2
                        while acc2 >= 1.0 and pend2:
                            pend2.pop(0)()
                            acc2 -= 1.0
                while pend2:
                    pend2.pop(0)()

            # epilogue: O-proj of the last chunk (attention already drained)
            for f in range(FT):
                o_strip(NCHUNK - 1, f)

    return nc


# Opcodes whose encodings accept multiple sync waits. On TRN2 every TPB
# engine instruction (and the DMA pseudo-instruction) takes at most ONE
# wait, so surplus waits are split into standalone EventSemaphore
# instructions spliced just before the offender (same engine stream =>
# identical semantics).
_WAIT_BUDGET = {}


def _split_waits_json(bir: bytes) -> bytes:
    import orjson

    j = orjson.loads(bir)
    ctr = 0
    for fn in j["functions"]:
        for blk in fn["blocks"]:
            out = []
            for ins in blk["instructions"]:
                si = ins.get("sync_info")
                waits = (si or {}).get("on_wait") or []
                budget = _WAIT_BUDGET.get(ins.get("opcode"), 1)
                if len(waits) > budget:
                    for w in waits[:-budget]:
                        ctr += 1
                        out.append(
                            {
                                "debug": ins.get("debug", 0),
                                "engine": ins["engine"],
                                "ins": [],
                                "name": f"Wsplit-{ctr}",
                                "opcode": "EventSemaphore",
                                "outs": [],
                                "sync_info": {"on_update": [], "on_wait": [w]},
                            }
                        )
                    si["on_wait"] = waits[-budget:]
                out.append(ins)
            blk["instructions"] = out
    return orjson.dumps(j)


def _install_ntff_shim():
    """This image's antenv lacks axon_hooks; provide it so trace=True works."""
    import sys, types

    if "antenv.axon_hooks" in sys.modules:
        return
    mod = types.ModuleType("antenv.axon_hooks")
    mod._hook = None

    def set_axon_ntff_profile_hook(h):
        mod._hook = h

    def get_axon_ntff_profile_hook():
        return mod._hook

    mod.set_axon_ntff_profile_hook = set_axon_ntff_profile_hook
    mod.get_axon_ntff_profile_hook = get_axon_ntff_profile_hook
    sys.modules["antenv.axon_hooks"] = mod
    try:
        import antenv

        antenv.axon_hooks = mod
    except ImportError:
        pass
    try:
        from trn_agent_boot.trn_boot import _ntff_profile_via_ctypes

        mod.set_axon_ntff_profile_hook(
            _ntff_profile_via_ctypes("/opt/axon/libaxon_pjrt.so")
        )
    except Exception as e:  # degrade: tracing skipped, run still works
        print(f"ntff shim: hook registration failed: {e}")


def _host_inputs(x, Wq, bq, Wk, bk, Wv, bv, Wo, bo, c_scale):
    """Build per-core in_maps (host-side shard + transpose + bf16 cast)."""
    bf = ml_dtypes.bfloat16
    xf = np.ascontiguousarray(np.asarray(x, np.float32).reshape(T_TOT, HID))

    def tile_w(W):  # w_h[f*128+p, kt*128+m] = W[kt*128+p, f*128+m]
        Wb = np.asarray(W, np.float32).astype(bf)
        return np.ascontiguousarray(
            Wb.reshape(KT, 128, FT, 128).transpose(2, 1, 0, 3).reshape(HID, HID)
        )

    brows = np.concatenate(
        [np.asarray(v, np.float32) for v in (bq, bk, bv, bo)]
    ).astype(bf).reshape(1, 4 * HID)

    scale = float(np.asarray(c_scale, np.float32).reshape(-1)[0]) / np.sqrt(D)
    scv = np.full((128, 1), scale, np.float32)

    # rank-9 additive mask: M[(h,t'),(g,t)] = 0 if t==t' else -a^2
    a = np.float32(np.sqrt(1e5))
    mrows = np.zeros((9, 128), bf)
    mcols = np.zeros((9, 512), bf)
    mrows[0, :] = a
    mcols[0, :] = -a
    for tau in range(GRP):
        for h in range(H):
            mrows[1 + tau, h * GRP + tau] = a
        for qd in range(QUAD):
            for g in range(H):
                mcols[1 + tau, qd * 128 + g * GRP + tau] = a
    ones_b = np.ones((128, 512), bf)
    ident = np.eye(128, dtype=bf)

    shared = dict(
        wq=tile_w(Wq), wk=tile_w(Wk), wo=tile_w(Wo), wv=tile_w(Wv),
        brows=brows, scv=scv, mrows=mrows, mcols=mcols, ones=ones_b,
        ident=ident,
    )
    in_maps = []
    for i in range(NCORES):
        xT_i = np.ascontiguousarray(xf[i * T : (i + 1) * T].T.astype(bf))
        in_maps.append(dict(xT=xT_i, **shared))
    return in_maps


def _assemble(results):
    outs = []
    for i in range(NCORES):
        outs.append(np.asarray(results[i]["outT"], np.float32).T)  # [T, HID]
    return np.concatenate(outs, axis=0).reshape(B, S, HID)


def run(inputs: dict, trace: bool = False):
    """Compile + execute on 8 cores; returns (output, BassKernelResults)."""
    from concourse.bass_utils import run_bass_kernel_spmd

    if trace:
        _install_ntff_shim()
    wb = any(
        np.any(np.asarray(inputs[k], np.float32) != 0.0)
        for k in ("bq", "bk", "bv", "bo")
    )
    nc = build_nc(with_bias=wb)
    _orig_tjb = nc.to_json_bytes
    nc.to_json_bytes = lambda: _split_waits_json(_orig_tjb())
    in_maps = _host_inputs(**inputs)
    res = run_bass_kernel_spmd(
        nc, in_maps, core_ids=list(range(NCORES)), trace=trace
    )
    return _assemble(res.results), res


def kernel(**inputs) -> np.ndarray:
    out, _ = run(inputs, trace=False)
    return out
